# revision 1
# baseline (speedup 1.0000x reference)
"""DenseGATv2 layer on 8 Trainium2 NeuronCores (Bass/Tile).

Math: the reference computes, per head,
    e[i,j]  = leaky_relu(s_i[i] + s_j[j], 0.2)   (s_i = h@a_src, s_j = h@a_dst)
    attn    = softmax_j(where(adj[i,j], e, -9e15))
    out[i]  = attn @ h
Since exp is monotonic and softmax is scale-invariant per row i:
    exp(leaky_relu(s_i+s_j)) * exp(-0.2 s_i) = max(exp(s_j + 0.8 s_i), exp(0.2 s_j))
and the row-constant exp(-0.2 s_i) cancels in the softmax normalization.  With
per-node precomputes rep_i = exp(0.8 s_i) (replicated across partitions),
rv_j = exp(s_j) and v_j = exp(0.2 s_j) (per-partition scalars), the whole
masked softmax numerator for one (j-chunk, head) tile is:
    P'[j,i] = max(rep_i * rv_j, v_j)        one tensor_scalar   (bf16, 4x)
    Pm      = P' * mask[j,i]                one tensor_tensor   (bf16, 2x),
                                            4 heads stacked against a
                                            stride-0-repeat mask AP
— no dense exp/leaky passes on ScalarE at all.  An appended ones-column in the
aggregation operand yields the softmax denominator inside the same PE matmuls
that aggregate h (attention tile stationary, so the output lands
destination-rows-on-partitions and phase 2 is just reciprocal + scale).

Sharding: destination rows i split across 8 cores (512 rows each); every core
computes the full h = x @ [W | W@a_src | W@a_dst] locally (one 128-deep matmul
per j-chunk) and reduces over all 4096 source nodes j for its own rows.

Trn2 scheduling notes: walrus allows at most ONE hardware sync-wait per
engine instruction (extras must be legalized into EventSemaphore ops by
Bacc.finalize, which this kernel relies on).  To keep that legalization
cheap the kernel also ships all bulk inputs as a single concatenated
tensor (one DMA -> one queue semaphore) and drains h PSUM with one engine.
PSUM output accumulators are pre-zeroed with memset and accumulated with
start=False throughout: interleaved per-head accumulation regions sharing
a PSUM bank corrupt each other's first contribution when start=True zeroing
is used per region (observed on HW: last-written head exact, others short).
"""

import os

import numpy as np
import ml_dtypes

import concourse.bass as bass
import concourse.tile as tile
from concourse.bacc import Bacc
from concourse import mybir
from concourse.bass_utils import run_bass_kernel_spmd

bf16 = ml_dtypes.bfloat16

N, IN_DIM, HEADS, OUT_DIM = 4096, 128, 4, 64
NCORES, ROWS = 8, N // 8          # 512 dest rows per core
P = 128                           # partitions
C = N // P                        # 32 j-chunks
OWNC = ROWS // P                  # 4 own i-chunks per core
COLS = 2 * IN_DIM + 2 * HEADS     # 264 = 256 h cols + 4 s_src + 4 s_dst
DAUG = OUT_DIM + 1                # 65: head h-slice + ones column
BULK = ROWS + COLS + N            # xownT | W_aug | xT columns

_cache = {}


def _build_bass(repeat=1, hw_loop=False):
    nc = Bacc()
    f32 = mybir.dt.float32
    f16 = mybir.dt.float16
    bfl = mybir.dt.bfloat16
    Act = mybir.ActivationFunctionType
    Alu = mybir.AluOpType

    bulk = nc.declare_dram_parameter("bulk", [P, BULK], f32, isOutput=False)
    maskT = nc.declare_dram_parameter("maskT", [N, ROWS], bfl, isOutput=False)
    out = nc.declare_dram_parameter("out", [ROWS, HEADS * OUT_DIM], f32, isOutput=True)
    riT_dram = nc.dram_tensor("riT_scratch", [OWNC * HEADS, P], bfl)

    with tile.TileContext(nc) as tc:
        with (
            tc.tile_pool(name="consts", bufs=1) as consts,
            tc.tile_pool(name="hb", bufs=C) as hb_pool,
            tc.tile_pool(name="vr", bufs=C) as vr_pool,
            tc.tile_pool(name="mask", bufs=8) as mask_pool,
            tc.tile_pool(name="tt", bufs=4) as t_pool,
            tc.tile_pool(name="pm", bufs=4) as pm_pool,
            tc.tile_pool(name="fin", bufs=4) as fin_pool,
            tc.tile_pool(name="psout", bufs=1, space="PSUM") as ps_out_pool,
            tc.tile_pool(name="ps_h", bufs=3, space="PSUM") as ps_h_pool,
            tc.tile_pool(name="ps_s", bufs=1, space="PSUM") as ps_s_pool,
        ):
          import contextlib
          loop_ctx = (tc.For_i(0, repeat, 1,
                               hint_engines=tuple(mybir.EngineType(e) for e in
                                                  ("PE", "DVE", "Activation", "SP", "Pool")))
                      if hw_loop else contextlib.nullcontext())
          with loop_ctx:
           for _rep in range(1 if hw_loop else repeat):
            # per-own-chunk output accumulators: claim PSUM banks first so they
            # are never aliased with the h-matmul banks (no cross-pool WAW).
            ps_out = [ps_out_pool.tile([P, HEADS, DAUG], f32, tag=f"po{k}", name=f"ps_out{k}")
                      for k in range(OWNC)]
            for k in range(OWNC):
                nc.vector.memset(ps_out[k][:, :, :], 0.0)

            if os.environ.get("GAT_WARM", "1") == "1":
                # pre-warm the ACT exp table set while input DMAs run
                warm = consts.tile([1, 1], f32, tag="warm")
                nc.vector.memset(warm, 0.0)
                nc.scalar.activation(warm, warm, Act.Exp)

            # ---- all bulk inputs in ONE DMA -> one queue semaphore
            sb_bulk = consts.tile([P, BULK], f32, tag="sb_bulk")
            nc.sync.dma_start(out=sb_bulk[:, 0:ROWS + COLS], in_=bulk[:, 0:ROWS + COLS])
            nc.sync.dma_start(out=sb_bulk[:, ROWS + COLS:BULK], in_=bulk[:, ROWS + COLS:BULK])
            sb_xown = sb_bulk[:, 0:ROWS]
            sb_W = sb_bulk[:, ROWS:ROWS + COLS]
            sb_xT = sb_bulk[:, ROWS + COLS:BULK]
            w_sd = sb_bulk[:, ROWS + 2 * IN_DIM:ROWS + 2 * IN_DIM + HEADS]

            # ---- phase 0b: r_i = exp(0.8 s_src) for own rows, replicated
            # across partitions via DMA transpose + DRAM-bounce broadcast.
            ps_sown = ps_s_pool.tile([P, COLS], f32, tag="ps_s", name="ps_sown")
            for oc in range(OWNC):
                nc.tensor.matmul(
                    ps_sown[:, oc * HEADS:(oc + 1) * HEADS],
                    sb_xown[:, oc * P:(oc + 1) * P], w_sd,
                    start=True, stop=True,
                )
            vown = consts.tile([P, P], bfl, tag="vown")
            nc.vector.memset(vown, 0.0)
            nc.scalar.activation(vown[:, 0:OWNC * HEADS], ps_sown[:, 0:OWNC * HEADS],
                                 Act.Exp, scale=0.8)
            vT = consts.tile([P, P], bfl, tag="vT")
            nc.sync.dma_start(out=vT, in_=vown, transpose=True)
            nc.sync.dma_start(out=riT_dram[:, :], in_=vT[0:OWNC * HEADS, :])
            sb_rep = consts.tile([P, HEADS, ROWS], bfl, tag="sb_rep")
            base = riT_dram[:, :]
            if os.environ.get("GAT_BCAST", "new") == "new":
                for hd in range(HEADS):
                    bcast = bass.AP(tensor=base.tensor, offset=base.offset + hd * P,
                                    ap=[[0, P], [HEADS * P, OWNC], [1, P]])
                    nc.sync.dma_start(
                        out=sb_rep[:, hd, :].rearrange("p (oc t) -> p oc t", oc=OWNC),
                        in_=bcast)
            else:
                for hd in range(HEADS):
                    for oc in range(OWNC):
                        row = riT_dram[oc * HEADS + hd:oc * HEADS + hd + 1, :]
                        b = bass.AP(tensor=row.tensor, offset=row.offset,
                                    ap=[[0, P], row.ap[-1]])
                        nc.sync.dma_start(out=sb_rep[:, hd, oc * P:(oc + 1) * P], in_=b)

            # ---- phase 0c: h_aug per j-chunk; PSUM drained by VectorE only
            hb = []
            vr = []
            for c in range(C):
                ps_h = ps_h_pool.tile([P, COLS], f32, tag="ps_h")
                nc.tensor.matmul(ps_h, sb_xT[:, c * P:(c + 1) * P], sb_W,
                                 start=True, stop=True)
                hb_c = hb_pool.tile([P, HEADS, DAUG], bfl, tag="hb")
                nc.vector.memset(hb_c[:, :, OUT_DIM:DAUG], 1.0)
                nc.scalar.activation(
                    hb_c[:, :, 0:OUT_DIM],
                    ps_h[:, 0:2 * IN_DIM].rearrange("p (h d) -> p h d", h=HEADS),
                    Act.Copy,
                )
                vr_c = vr_pool.tile([P, 2, HEADS], f32, tag="vr")
                nc.scalar.activation(vr_c[:, 0, :], ps_h[:, 2 * IN_DIM + HEADS:COLS],
                                     Act.Exp, scale=0.2)
                nc.scalar.activation(vr_c[:, 1, :], ps_h[:, 2 * IN_DIM + HEADS:COLS],
                                     Act.Exp, scale=1.0)
                hb.append(hb_c)
                vr.append(vr_c)

            # ---- phase 1: hot loop over j-chunks
            for c in range(C):
                mask_c = mask_pool.tile([P, ROWS], bfl, tag="mask")
                nc.sync.dma_start(out=mask_c, in_=maskT[c * P:(c + 1) * P, :])
                t_all = t_pool.tile([P, HEADS, ROWS], bfl, tag="T")
                for hd in range(HEADS):
                    nc.vector.tensor_scalar(
                        out=t_all[:, hd, :], in0=sb_rep[:, hd, :],
                        scalar1=vr[c][:, 1, hd:hd + 1],
                        scalar2=vr[c][:, 0, hd:hd + 1],
                        op0=Alu.mult, op1=Alu.max,
                    )
                pm_all = pm_pool.tile([P, HEADS, ROWS], bfl, tag="pm")
                for hd in range(HEADS):
                    nc.vector.tensor_tensor(out=pm_all[:, hd, :],
                                            in0=t_all[:, hd, :], in1=mask_c,
                                            op=Alu.mult)
                for hd in range(HEADS):
                    for k in range(OWNC):
                        nc.tensor.matmul(
                            ps_out[k][:, hd, :],
                            pm_all[:, hd, k * P:(k + 1) * P], hb[c][:, hd, :],
                            start=False, stop=(c == C - 1),
                            skip_group_check=True,
                        )

            # ---- phase 2: normalize + store (dest rows already on partitions)
            for k in range(OWNC):
                out_k = fin_pool.tile([P, HEADS, OUT_DIM], f32, tag="outk")
                for hd in range(HEADS):
                    rcp = fin_pool.tile([P, 1], f32, tag="rcp")
                    nc.vector.reciprocal(rcp, ps_out[k][:, hd, OUT_DIM:DAUG])
                    if os.environ.get("GAT_FIN", "act") == "act":
                        nc.scalar.activation(
                            out_k[:, hd, :], ps_out[k][:, hd, 0:OUT_DIM],
                            Act.Copy, scale=rcp,
                        )
                    else:
                        nc.vector.tensor_scalar(
                            out=out_k[:, hd, :], in0=ps_out[k][:, hd, 0:OUT_DIM],
                            scalar1=rcp, scalar2=None, op0=Alu.mult,
                        )
                nc.sync.dma_start(
                    out=out[k * P:(k + 1) * P, :].rearrange("p (h d) -> p h d", h=HEADS),
                    in_=out_k,
                )
    nc.finalize()
    return nc


def _prep_in_maps(x, adj_mask, W_lin, a_src, a_dst):

    W_lin = np.asarray(W_lin, np.float32)
    W3 = W_lin.reshape(IN_DIM, HEADS, OUT_DIM).astype(np.float64)
    W_src = (W3 @ np.asarray(a_src, np.float64)).astype(np.float32)
    W_dst = (W3 @ np.asarray(a_dst, np.float64)).astype(np.float32)
    W_aug = np.concatenate([W_lin, W_src, W_dst], axis=1)
    x = np.asarray(x, np.float32)
    xT = np.ascontiguousarray(x.T)
    adj = np.asarray(adj_mask, bool)
    maskT = np.where(adj.T, np.float32(1.0), np.float32(0.0)).astype(bf16)

    in_maps = []
    for core in range(NCORES):
        sl = slice(core * ROWS, (core + 1) * ROWS)
        bulk = np.ascontiguousarray(
            np.concatenate([xT[:, sl], W_aug, xT], axis=1))
        in_maps.append({
            "bulk": bulk,
            "maskT": np.ascontiguousarray(maskT[:, sl]),
        })

    return in_maps


def kernel(x, adj_mask, W_lin, a_src, a_dst):
    if "nc" not in _cache:
        _cache["nc"] = _build_bass()
    nc = _cache["nc"]
    in_maps = _prep_in_maps(x, adj_mask, W_lin, a_src, a_dst)
    res = run_bass_kernel_spmd(nc, in_maps, core_ids=list(range(NCORES)))
    outs = [r["out"] for r in res.results]
    return np.concatenate(outs, axis=0).astype(np.float32)



# revision 3
# speedup vs baseline: 5.2252x; 5.2252x over previous
"""DenseGATv2 layer on 8 Trainium2 NeuronCores (Bass/Tile) — v2.

Same math as the baseline (see derivation below) but restructured to minimize
STATIC instruction count, which is what the per-invocation cost of this
backend is proportional to (program load/processing dominates; dynamic
execution is ~100us and negligible).

Math: per head,
    e[i,j]  = leaky_relu(s_i[i] + s_j[j], 0.2)   (s_i = h@a_src, s_j = h@a_dst)
    attn    = softmax_j(where(adj[i,j], e, -9e15))
    out[i]  = attn @ h
Using exp monotonicity and softmax row-scale invariance (multiply row i by
exp(-0.2 s_i)):
    numerator P'[j,i] = max(rep_i * rv_j, v_j) * mask[j,i]
with rep_i = exp(0.8 s_i), rv_j = exp(s_j), v_j = exp(0.2 s_j).

Key structural choices vs the old kernel:
  - Aggregation is FLIPPED: stationary = h_aug chunk [128j, 65] per head,
    moving = P' [128j, 512i] -> PSUM out [65, 512] accumulated over all 32
    j-chunks. 4 matmuls/chunk instead of 16 (out rows = head dims + ones row
    giving the softmax denominator per column i).
  - Heads stacked in DVE ops with broadcast APs: 3 tensor_tensor per chunk
    (mult rv, max v, mult mask) on [P, 4, 512] instead of 8 per-head ops.
  - h for 4 chunks lands in one 4-bank PSUM tile, drained with 1 grouped exp
    (+ per-chunk or grouped copies).
  - One DMA loads the whole transposed mask slice; one transposed-AP DMA
    stores the whole normalized output (no on-device transpose dance).
  - W_aug ships [W | 0.8*W_src | W_dst | 0.2*W_dst] so every exp is a plain
    table lookup and the s-columns drop out of the h matmul for free.
"""

import os
import contextlib

import numpy as np
import ml_dtypes

import concourse.bass as bass
import concourse.tile as tile
from concourse.bacc import Bacc
from concourse import mybir
from concourse.bass_utils import run_bass_kernel_spmd

bf16 = ml_dtypes.bfloat16

N, IN_DIM, HEADS, OUT_DIM = 4096, 128, 4, 64
NCORES, ROWS = 8, N // 8          # 512 dest rows per core
P = 128                           # partitions
C = N // P                        # 32 j-chunks
OWNC = ROWS // P                  # 4 own i-chunks per core
DAUG = OUT_DIM + 1                # 65: head h-slice + ones column
WCOLS = 2 * IN_DIM + 3 * HEADS    # 268 = 256 h | 4x 0.8Wsrc | 4x Wdst | 4x 0.2Wdst
BULK = ROWS + WCOLS + N           # xownT | W_aug | xT columns
GRP = 4                           # h chunks per PSUM drain group

_cache = {}


def _build_bass(repeat=1, hw_loop=False):
    nc = Bacc()
    f32 = mybir.dt.float32
    bfl = mybir.dt.bfloat16
    Act = mybir.ActivationFunctionType
    Alu = mybir.AluOpType
    group_cp = os.environ.get("GAT_GROUPCP", "1") == "1"
    group_tt = int(os.environ.get("GAT_GROUPTT", "4"))

    bulk = nc.declare_dram_parameter("bulk", [P, BULK], f32, isOutput=False)
    maskT = nc.declare_dram_parameter("maskT", [N, ROWS], bfl, isOutput=False)
    # out stays in the flipped [d, (hd, i)] layout; the host transposes.
    out = nc.declare_dram_parameter("out", [OUT_DIM, HEADS * ROWS], f32, isOutput=True)
    riT_dram = nc.dram_tensor("riT_scratch", [OWNC * HEADS, P], bfl)
    rcp_scr = nc.dram_tensor("rcp_scr", [1, HEADS * ROWS], f32)

    with tile.TileContext(nc) as tc:
        with (
            tc.tile_pool(name="consts", bufs=1) as consts,
            tc.tile_pool(name="tt", bufs=2) as t_pool,
            tc.tile_pool(name="pst", bufs=1, space="PSUM") as pst_pool,
            tc.tile_pool(name="ps4", bufs=1, space="PSUM") as ps4_pool,
        ):
          loop_ctx = (tc.For_i(0, repeat, 1,
                               hint_engines=tuple(mybir.EngineType(e) for e in
                                                  ("PE", "DVE", "Activation", "SP", "Pool")))
                      if hw_loop else contextlib.nullcontext())
          with loop_ctx:
           for _rep in range(1 if hw_loop else repeat):
            # ---- persistent tiles
            sb_bulk = consts.tile([P, BULK], f32, tag="bulk")
            mask_all = consts.tile([P, C, ROWS], bfl, tag="mask")
            hb_all = consts.tile([P, C, HEADS, DAUG], bfl, tag="hb")
            vr_all = consts.tile([P, C, 3 * HEADS], f32, tag="vr")
            rep_t = consts.tile([P, HEADS, ROWS], bfl, tag="rep")

            # ---- input DMAs (1 bulk + 1 mask)
            nc.sync.dma_start(out=sb_bulk[:, :], in_=bulk[:, :])
            sb_xown = sb_bulk[:, 0:ROWS]
            sb_W = sb_bulk[:, ROWS:ROWS + WCOLS]
            sb_xT = sb_bulk[:, ROWS + WCOLS:BULK]
            mbase = maskT[:, :]
            mask_ap = bass.AP(tensor=mbase.tensor, offset=mbase.offset,
                              ap=[[ROWS, P], [P * ROWS, C], [1, ROWS]])
            nc.sync.dma_start(out=mask_all[:, :, :], in_=mask_ap)

            # warm the ACT exp table while DMAs run
            warm = consts.tile([1, 1], f32, tag="warm")
            nc.vector.memset(warm, 0.0)
            nc.scalar.activation(warm, warm, Act.Exp)

            # ones column of h_aug (col 64 of every head block), written once
            nc.vector.memset(hb_all[:, :, :, OUT_DIM:DAUG], 1.0)

            # ---- PSUM claims: psT = flipped output accumulators (4 banks),
            # ps4 = 4-chunk h staging (4 banks). ps4 slot-0 slack cols hold
            # the own-row 0.8*s_src values (never overwritten: h writes only
            # cols 0:WCOLS of each slot).
            psT = pst_pool.tile([DAUG, HEADS, ROWS], f32, tag="psT")
            nc.vector.memset(psT[:, :, :], 0.0)
            ps4 = ps4_pool.tile([P, GRP, 512], f32, tag="ps4")

            # ---- rep_i = exp(0.8 s_src) for own rows, replicated across
            # partitions via SBUF transpose + DRAM-bounce broadcast (the
            # per-head read APs are the only DMA-legal form: <=3 dims with a
            # contiguous final dim).
            for oc in range(OWNC):
                nc.tensor.matmul(
                    ps4[:, 0, WCOLS + HEADS * oc:WCOLS + HEADS * (oc + 1)],
                    sb_xown[:, oc * P:(oc + 1) * P],
                    sb_W[:, 2 * IN_DIM:2 * IN_DIM + HEADS],
                    start=True, stop=True,
                )
            vown = consts.tile([P, P], bfl, tag="vown")
            nc.vector.memset(vown, 0.0)
            nc.scalar.activation(
                vown[:, 0:OWNC * HEADS],
                ps4[:, 0, WCOLS:WCOLS + OWNC * HEADS], Act.Exp)
            vT = consts.tile([P, P], bfl, tag="vT")
            nc.sync.dma_start(out=vT, in_=vown, transpose=True)
            nc.sync.dma_start(out=riT_dram[:, :], in_=vT[0:OWNC * HEADS, :])
            rbase = riT_dram[:, :]
            for hd in range(HEADS):
                bcast = bass.AP(tensor=rbase.tensor, offset=rbase.offset + hd * P,
                                ap=[[0, P], [HEADS * P, OWNC], [1, P]])
                nc.sync.dma_start(
                    out=rep_t[:, hd, :].rearrange("p (oc t) -> p oc t", oc=OWNC),
                    in_=bcast)

            # ---- h_aug for all chunks, 4 per PSUM group
            for g in range(C // GRP):
                for k in range(GRP):
                    c = g * GRP + k
                    nc.tensor.matmul(ps4[:, k, 0:WCOLS],
                                     sb_xT[:, c * P:(c + 1) * P], sb_W,
                                     start=True, stop=True)
                if group_cp:
                    nc.scalar.activation(
                        hb_all[:, g * GRP:(g + 1) * GRP, :, 0:OUT_DIM],
                        ps4[:, :, 0:2 * IN_DIM].rearrange(
                            "p k (h d) -> p k h d", h=HEADS),
                        Act.Copy)
                else:
                    for k in range(GRP):
                        c = g * GRP + k
                        nc.scalar.activation(
                            hb_all[:, c, :, 0:OUT_DIM],
                            ps4[:, k, 0:2 * IN_DIM].rearrange(
                                "p (h d) -> p h d", h=HEADS),
                            Act.Copy)
                nc.scalar.activation(
                    vr_all[:, g * GRP:(g + 1) * GRP, :],
                    ps4[:, :, 2 * IN_DIM:WCOLS],
                    Act.Exp)

            # ---- hot loop over j-chunks: 3 DVE ops + 4 matmuls per chunk
            for c0 in range(0, C, group_tt):
                gn = group_tt
                t1 = t_pool.tile([P, gn, HEADS, ROWS], bfl, tag="t1")
                t2 = t_pool.tile([P, gn, HEADS, ROWS], bfl, tag="t2")
                pm = t_pool.tile([P, gn, HEADS, ROWS], bfl, tag="pm")
                rep_b = rep_t[:, :, :].unsqueeze(1).broadcast_to(
                    (P, gn, HEADS, ROWS))
                rv_b = vr_all[:, c0:c0 + gn, HEADS:2 * HEADS].unsqueeze(
                    3).broadcast_to((P, gn, HEADS, ROWS))
                v_b = vr_all[:, c0:c0 + gn, 2 * HEADS:3 * HEADS].unsqueeze(
                    3).broadcast_to((P, gn, HEADS, ROWS))
                mask_b = mask_all[:, c0:c0 + gn, :].unsqueeze(2).broadcast_to(
                    (P, gn, HEADS, ROWS))
                nc.vector.tensor_tensor(out=t1[:, :, :, :], in0=rep_b,
                                        in1=rv_b, op=Alu.mult)
                nc.vector.tensor_tensor(out=t2[:, :, :, :], in0=t1[:, :, :, :],
                                        in1=v_b, op=Alu.max)
                nc.vector.tensor_tensor(out=pm[:, :, :, :], in0=t2[:, :, :, :],
                                        in1=mask_b, op=Alu.mult)
                for k in range(gn):
                    c = c0 + k
                    for hd in range(HEADS):
                        nc.tensor.matmul(
                            psT[:, hd, :],
                            hb_all[:, c, hd, :], pm[:, k, hd, :],
                            start=False, stop=(c == C - 1),
                            skip_group_check=True,
                        )

            # ---- normalize + store: reciprocal of the denominator row,
            # partition-broadcast it via DRAM bounce, scale, one transposed
            # store of the whole [512, 256] output.
            rcp_sb = consts.tile([1, HEADS * ROWS], f32, tag="rcp")
            nc.vector.reciprocal(
                rcp_sb[:, :],
                psT[OUT_DIM:DAUG, :, :].rearrange("p h i -> p (h i)"))
            cbase = rcp_scr[:, :]
            nc.sync.dma_start(out=rcp_scr[:, :], in_=rcp_sb[:, :])
            recb = consts.tile([OUT_DIM, HEADS, ROWS], f32, tag="recb")
            nc.sync.dma_start(
                out=recb[:, :, :],
                in_=bass.AP(tensor=cbase.tensor, offset=cbase.offset,
                            ap=[[0, OUT_DIM], [ROWS, HEADS], [1, ROWS]]))
            out_sb = consts.tile([OUT_DIM, HEADS, ROWS], f32, tag="osb")
            nc.vector.tensor_tensor(out=out_sb[:, :, :],
                                    in0=psT[0:OUT_DIM, :, :],
                                    in1=recb[:, :, :], op=Alu.mult)
            nc.sync.dma_start(
                out=out[:, :].rearrange("p (h i) -> p h i", h=HEADS),
                in_=out_sb[:, :, :])
    nc.finalize()
    return nc


def _prep_in_maps(x, adj_mask, W_lin, a_src, a_dst):
    W_lin = np.asarray(W_lin, np.float32)
    W3 = W_lin.reshape(IN_DIM, HEADS, OUT_DIM).astype(np.float64)
    W_src = (W3 @ np.asarray(a_src, np.float64)).astype(np.float32)
    W_dst = (W3 @ np.asarray(a_dst, np.float64)).astype(np.float32)
    W_aug = np.concatenate(
        [W_lin, 0.8 * W_src, W_dst, 0.2 * W_dst], axis=1)
    x = np.asarray(x, np.float32)
    xT = np.ascontiguousarray(x.T)
    adj = np.asarray(adj_mask, bool)
    maskT = np.where(adj.T, np.float32(1.0), np.float32(0.0)).astype(bf16)

    in_maps = []
    for core in range(NCORES):
        sl = slice(core * ROWS, (core + 1) * ROWS)
        blk = np.ascontiguousarray(
            np.concatenate([xT[:, sl], W_aug, xT], axis=1))
        in_maps.append({
            "bulk": blk,
            "maskT": np.ascontiguousarray(maskT[:, sl]),
        })
    return in_maps


def _post(results):
    outs = []
    for r in results:
        # device layout [d, (hd, i)] -> [i, (hd, d)]
        a = r["out"].reshape(OUT_DIM, HEADS, ROWS)
        outs.append(np.ascontiguousarray(a.transpose(2, 1, 0)).reshape(
            ROWS, HEADS * OUT_DIM))
    return np.concatenate(outs, axis=0).astype(np.float32)


def kernel(x, adj_mask, W_lin, a_src, a_dst):
    if "nc" not in _cache:
        _cache["nc"] = _build_bass()
    nc = _cache["nc"]
    in_maps = _prep_in_maps(x, adj_mask, W_lin, a_src, a_dst)
    res = run_bass_kernel_spmd(nc, in_maps, core_ids=list(range(NCORES)))
    return _post(res.results)


# revision 4
# speedup vs baseline: 15.4464x; 2.9561x over previous
"""DenseGATv2 layer on 8 Trainium2 NeuronCores (Bass/Tile) — v2.

Same math as the baseline (see derivation below) but restructured to minimize
STATIC instruction count, which is what the per-invocation cost of this
backend is proportional to (program load/processing dominates; dynamic
execution is ~100us and negligible).

Math: per head,
    e[i,j]  = leaky_relu(s_i[i] + s_j[j], 0.2)   (s_i = h@a_src, s_j = h@a_dst)
    attn    = softmax_j(where(adj[i,j], e, -9e15))
    out[i]  = attn @ h
Using exp monotonicity and softmax row-scale invariance (multiply row i by
exp(-0.2 s_i)):
    numerator P'[j,i] = max(rep_i * rv_j, v_j) * mask[j,i]
with rep_i = exp(0.8 s_i), rv_j = exp(s_j), v_j = exp(0.2 s_j).

Key structural choices vs the old kernel:
  - Aggregation is FLIPPED: stationary = h_aug chunk [128j, 65] per head,
    moving = P' [128j, 512i] -> PSUM out [65, 512] accumulated over all 32
    j-chunks. 4 matmuls/chunk instead of 16 (out rows = head dims + ones row
    giving the softmax denominator per column i).
  - Heads stacked in DVE ops with broadcast APs: 3 tensor_tensor per chunk
    (mult rv, max v, mult mask) on [P, 4, 512] instead of 8 per-head ops.
  - h for 4 chunks lands in one 4-bank PSUM tile, drained with 1 grouped exp
    (+ per-chunk or grouped copies).
  - One DMA loads the whole transposed mask slice; one transposed-AP DMA
    stores the whole normalized output (no on-device transpose dance).
  - W_aug ships [W | 0.8*W_src | W_dst | 0.2*W_dst] so every exp is a plain
    table lookup and the s-columns drop out of the h matmul for free.
"""

import os
import contextlib

import numpy as np
import ml_dtypes

import concourse.bass as bass
import concourse.tile as tile
from concourse.bacc import Bacc
from concourse import mybir
from concourse.bass_utils import run_bass_kernel_spmd

bf16 = ml_dtypes.bfloat16

N, IN_DIM, HEADS, OUT_DIM = 4096, 128, 4, 64
NCORES, ROWS = 8, N // 8          # 512 dest rows per core
P = 128                           # partitions
C = N // P                        # 32 j-chunks
OWNC = ROWS // P                  # 4 own i-chunks per core
DAUG = OUT_DIM + 1                # 65: head h-slice + ones column
WCOLS = 2 * IN_DIM + 3 * HEADS    # 268 = 256 h | 4x 0.8Wsrc | 4x Wdst | 4x 0.2Wdst
BULK = ROWS + WCOLS + N           # xownT | W_aug | xT columns
GRP = 4                           # h chunks per PSUM drain group

_cache = {}


def _build_bass(repeat=1, hw_loop=False):
    nc = Bacc()
    f32 = mybir.dt.float32
    bfl = mybir.dt.bfloat16
    Act = mybir.ActivationFunctionType
    Alu = mybir.AluOpType
    group_cp = os.environ.get("GAT_GROUPCP", "1") == "1"
    group_tt = int(os.environ.get("GAT_GROUPTT", "8"))

    bulk = nc.declare_dram_parameter("bulk", [P, BULK], f32, isOutput=False)
    maskT = nc.declare_dram_parameter("maskT", [N, ROWS], bfl, isOutput=False)
    # out stays in the flipped [d, (hd, i)] layout; the host transposes.
    out = nc.declare_dram_parameter("out", [OUT_DIM, HEADS * ROWS], f32, isOutput=True)
    riT_dram = nc.dram_tensor("riT_scratch", [OWNC * HEADS, P], bfl)
    rcp_scr = nc.dram_tensor("rcp_scr", [1, HEADS * ROWS], f32)

    with tile.TileContext(nc) as tc:
        with (
            tc.tile_pool(name="consts", bufs=1) as consts,
            tc.tile_pool(name="tt", bufs=2 if int(os.environ.get("GAT_GROUPTT", "8")) <= 4 else 1) as t_pool,
            tc.tile_pool(name="pst", bufs=1, space="PSUM") as pst_pool,
            tc.tile_pool(name="ps4", bufs=1, space="PSUM") as ps4_pool,
        ):
          loop_ctx = (tc.For_i(0, repeat, 1,
                               hint_engines=tuple(mybir.EngineType(e) for e in
                                                  ("PE", "DVE", "Activation", "SP", "Pool")))
                      if hw_loop else contextlib.nullcontext())
          with loop_ctx:
           for _rep in range(1 if hw_loop else repeat):
            # ---- persistent tiles
            sb_bulk = consts.tile([P, BULK], f32, tag="bulk")
            mask_all = consts.tile([P, C, ROWS], bfl, tag="mask")
            hb_all = consts.tile([P, C, HEADS, DAUG], bfl, tag="hb")
            vr_all = consts.tile([P, C, 3 * HEADS], f32, tag="vr")
            rep_t = consts.tile([P, HEADS, ROWS], bfl, tag="rep")

            # ---- input DMAs (1 bulk + 1 mask)
            nc.sync.dma_start(out=sb_bulk[:, :], in_=bulk[:, :])
            sb_xown = sb_bulk[:, 0:ROWS]
            sb_W = sb_bulk[:, ROWS:ROWS + WCOLS]
            sb_xT = sb_bulk[:, ROWS + WCOLS:BULK]
            mbase = maskT[:, :]
            mask_ap = bass.AP(tensor=mbase.tensor, offset=mbase.offset,
                              ap=[[ROWS, P], [P * ROWS, C], [1, ROWS]])
            nc.sync.dma_start(out=mask_all[:, :, :], in_=mask_ap)

            # ones column of h_aug (col 64 of every head block), written once
            nc.vector.memset(hb_all[:, :, :, OUT_DIM:DAUG], 1.0)

            # ---- PSUM claims: psT = flipped output accumulators (4 banks),
            # ps4 = 4-chunk h staging (4 banks). ps4 slot-0 slack cols hold
            # the own-row 0.8*s_src values (never overwritten: h writes only
            # cols 0:WCOLS of each slot).
            psT = pst_pool.tile([DAUG, HEADS, ROWS], f32, tag="psT")
            nc.vector.memset(psT[:, :, :], 0.0)
            ps4 = ps4_pool.tile([P, GRP, 512], f32, tag="ps4")

            # ---- rep_i = exp(0.8 s_src) for own rows, replicated across
            # partitions via SBUF transpose + DRAM-bounce broadcast (the
            # per-head read APs are the only DMA-legal form: <=3 dims with a
            # contiguous final dim).
            for oc in range(OWNC):
                nc.tensor.matmul(
                    ps4[:, 0, WCOLS + HEADS * oc:WCOLS + HEADS * (oc + 1)],
                    sb_xown[:, oc * P:(oc + 1) * P],
                    sb_W[:, 2 * IN_DIM:2 * IN_DIM + HEADS],
                    start=True, stop=True,
                )
            vown = consts.tile([P, P], bfl, tag="vown")
            nc.vector.memset(vown, 0.0)
            nc.scalar.activation(
                vown[:, 0:OWNC * HEADS],
                ps4[:, 0, WCOLS:WCOLS + OWNC * HEADS], Act.Exp)
            vT = consts.tile([P, P], bfl, tag="vT")
            nc.sync.dma_start(out=vT, in_=vown, transpose=True)
            nc.sync.dma_start(out=riT_dram[:, :], in_=vT[0:OWNC * HEADS, :])
            rbase = riT_dram[:, :]
            for hd in range(HEADS):
                bcast = bass.AP(tensor=rbase.tensor, offset=rbase.offset + hd * P,
                                ap=[[0, P], [HEADS * P, OWNC], [1, P]])
                nc.sync.dma_start(
                    out=rep_t[:, hd, :].rearrange("p (oc t) -> p oc t", oc=OWNC),
                    in_=bcast)

            # ---- h_aug for all chunks, 4 per PSUM group
            for g in range(C // GRP):
                for k in range(GRP):
                    c = g * GRP + k
                    nc.tensor.matmul(ps4[:, k, 0:WCOLS],
                                     sb_xT[:, c * P:(c + 1) * P], sb_W,
                                     start=True, stop=True)
                if group_cp:
                    nc.scalar.activation(
                        hb_all[:, g * GRP:(g + 1) * GRP, :, 0:OUT_DIM],
                        ps4[:, :, 0:2 * IN_DIM].rearrange(
                            "p k (h d) -> p k h d", h=HEADS),
                        Act.Copy)
                else:
                    for k in range(GRP):
                        c = g * GRP + k
                        nc.scalar.activation(
                            hb_all[:, c, :, 0:OUT_DIM],
                            ps4[:, k, 0:2 * IN_DIM].rearrange(
                                "p (h d) -> p h d", h=HEADS),
                            Act.Copy)
                nc.scalar.activation(
                    vr_all[:, g * GRP:(g + 1) * GRP, :],
                    ps4[:, :, 2 * IN_DIM:WCOLS],
                    Act.Exp)

            # ---- hot loop over j-chunks: 3 DVE ops + 4 matmuls per chunk
            for c0 in range(0, C, group_tt):
                gn = group_tt
                t1 = t_pool.tile([P, gn, HEADS, ROWS], bfl, tag="t1")
                t2 = t_pool.tile([P, gn, HEADS, ROWS], bfl, tag="t2")
                pm = t_pool.tile([P, gn, HEADS, ROWS], bfl, tag="pm")
                rep_b = rep_t[:, :, :].unsqueeze(1).broadcast_to(
                    (P, gn, HEADS, ROWS))
                rv_b = vr_all[:, c0:c0 + gn, HEADS:2 * HEADS].unsqueeze(
                    3).broadcast_to((P, gn, HEADS, ROWS))
                v_b = vr_all[:, c0:c0 + gn, 2 * HEADS:3 * HEADS].unsqueeze(
                    3).broadcast_to((P, gn, HEADS, ROWS))
                mask_b = mask_all[:, c0:c0 + gn, :].unsqueeze(2).broadcast_to(
                    (P, gn, HEADS, ROWS))
                nc.vector.tensor_tensor(out=t1[:, :, :, :], in0=rep_b,
                                        in1=rv_b, op=Alu.mult)
                nc.vector.tensor_tensor(out=t2[:, :, :, :], in0=t1[:, :, :, :],
                                        in1=v_b, op=Alu.max)
                nc.vector.tensor_tensor(out=pm[:, :, :, :], in0=t2[:, :, :, :],
                                        in1=mask_b, op=Alu.mult)
                for k in range(gn):
                    c = c0 + k
                    for hd in range(HEADS):
                        nc.tensor.matmul(
                            psT[:, hd, :],
                            hb_all[:, c, hd, :], pm[:, k, hd, :],
                            start=False, stop=(c == C - 1),
                            skip_group_check=True,
                        )

            # ---- normalize + store: reciprocal of the denominator row,
            # partition-broadcast it via DRAM bounce, scale, one transposed
            # store of the whole [512, 256] output.
            rcp_sb = consts.tile([1, HEADS * ROWS], f32, tag="rcp")
            nc.vector.reciprocal(
                rcp_sb[:, :],
                psT[OUT_DIM:DAUG, :, :].rearrange("p h i -> p (h i)"))
            cbase = rcp_scr[:, :]
            nc.sync.dma_start(out=rcp_scr[:, :], in_=rcp_sb[:, :])
            recb = consts.tile([OUT_DIM, HEADS, ROWS], f32, tag="recb")
            nc.sync.dma_start(
                out=recb[:, :, :],
                in_=bass.AP(tensor=cbase.tensor, offset=cbase.offset,
                            ap=[[0, OUT_DIM], [ROWS, HEADS], [1, ROWS]]))
            out_sb = consts.tile([OUT_DIM, HEADS, ROWS], f32, tag="osb")
            nc.vector.tensor_tensor(out=out_sb[:, :, :],
                                    in0=psT[0:OUT_DIM, :, :],
                                    in1=recb[:, :, :], op=Alu.mult)
            nc.sync.dma_start(
                out=out[:, :].rearrange("p (h i) -> p h i", h=HEADS),
                in_=out_sb[:, :, :])
    nc.finalize()
    return nc


def _prep_in_maps(x, adj_mask, W_lin, a_src, a_dst):
    W_lin = np.asarray(W_lin, np.float32)
    W3 = W_lin.reshape(IN_DIM, HEADS, OUT_DIM).astype(np.float64)
    W_src = (W3 @ np.asarray(a_src, np.float64)).astype(np.float32)
    W_dst = (W3 @ np.asarray(a_dst, np.float64)).astype(np.float32)
    W_aug = np.concatenate(
        [W_lin, 0.8 * W_src, W_dst, 0.2 * W_dst], axis=1)
    x = np.asarray(x, np.float32)
    xT = np.ascontiguousarray(x.T)
    adj = np.asarray(adj_mask, bool)
    maskT = np.where(adj.T, np.float32(1.0), np.float32(0.0)).astype(bf16)

    in_maps = []
    for core in range(NCORES):
        sl = slice(core * ROWS, (core + 1) * ROWS)
        blk = np.ascontiguousarray(
            np.concatenate([xT[:, sl], W_aug, xT], axis=1))
        in_maps.append({
            "bulk": blk,
            "maskT": np.ascontiguousarray(maskT[:, sl]),
        })
    return in_maps


def _post(results):
    outs = []
    for r in results:
        # device layout [d, (hd, i)] -> [i, (hd, d)]
        a = r["out"].reshape(OUT_DIM, HEADS, ROWS)
        outs.append(np.ascontiguousarray(a.transpose(2, 1, 0)).reshape(
            ROWS, HEADS * OUT_DIM))
    return np.concatenate(outs, axis=0).astype(np.float32)


def kernel(x, adj_mask, W_lin, a_src, a_dst):
    if "nc" not in _cache:
        _cache["nc"] = _build_bass()
    nc = _cache["nc"]
    in_maps = _prep_in_maps(x, adj_mask, W_lin, a_src, a_dst)
    res = run_bass_kernel_spmd(nc, in_maps, core_ids=list(range(NCORES)))
    return _post(res.results)
